# revision 40
# baseline (speedup 1.0000x reference)
import sys, os
sys.path.insert(0, "/opt/trn_rl_repo")
os.environ.setdefault("NEURON_RT_LOG_LEVEL", "WARNING")
import numpy as np
import ml_dtypes

import concourse.bass as bass
import concourse.bacc as bacc
import concourse.mybir as mybir
import concourse.tile as tile
from concourse import masks
from concourse.bass_utils import run_bass_kernel_spmd

dt = mybir.dt
bf16 = ml_dtypes.bfloat16
NC = 8


def ceil128(v):
    return (int(v) + 127) // 128 * 128


def build_host_data(x, edge_index, W1, b1, W2, b2, W3, b3, W4, b4, Wl, bl,
                    ch0=3200):
    """Partition the graph by destination node across 8 cores and build all
    per-core device input arrays plus the (core-uniform) segment layout.

    Sources are split into two chunks by their offset within the OWNER
    core's shard (off < ch0 vs off >= ch0).  Each chunk has its own
    rank-major all-gathered table, so the chunk-0 collective can fire as
    soon as the producing pass finishes its first 25 destination tiles."""
    N = x.shape[0]
    NPC = N // NC
    TPC = (NPC + 127) // 128
    NPAD = TPC * 128
    ch1 = NPC - ch0

    # self-loops are applied on-device via an identity matmul over the
    # core-local table shard; only real edges go through the gather path.
    src = edge_index[0].astype(np.int64)
    dst = edge_index[1].astype(np.int64)
    deg = (np.bincount(dst, minlength=N) + 1).astype(np.float32)

    core = dst // NPC
    dstl = dst - core * NPC
    tl = dstl >> 7
    dl128 = (dstl & 127).astype(np.float32)
    sowner = src // NPC
    soff = src - sowner * NPC
    h = (soff >= ch0).astype(np.int64)
    # index of the source row inside its chunk table (rank-major)
    cidx = np.where(h == 0, sowner * ch0 + soff,
                    sowner * ch1 + (soff - ch0))

    order = np.lexsort((cidx, h, tl, core))
    s_src = cidx[order]
    s_dl = dl128[order]

    key = (core * TPC + tl) * 2 + h
    cnt = np.bincount(key, minlength=NC * TPC * 2).reshape(NC, TPC, 2)
    m = np.maximum(cnt.max(axis=0), 0)
    m = ((m + 127) // 128 * 128).astype(np.int64)          # [TPC, 2]

    # segment table (uniform across cores). Stream layout is GROUP-major:
    # tiles are grouped G at a time; within a group all h=0 segments come
    # first (one gather base), then all h=1 segments. segs[t] keeps the
    # per-tile (h, pos, len) list for data placement.
    G = 4
    segs = [[] for _ in range(TPC)]
    groups = []          # list of (h -> list of (t, pos, len))
    pos = 0
    for g0 in range(0, TPC, G):
        tl = list(range(g0, min(g0 + G, TPC)))
        gmeta = {0: [], 1: []}
        for hh in (0, 1):
            for t in tl:
                L = int(m[t, hh])
                if L:
                    gmeta[hh].append((t, pos, L))
                    segs[t].append((hh, pos, L))
                    pos += L
        groups.append(gmeta)
    TOT = pos
    assert TOT % 128 == 0

    # boundaries of (core, tile, half) runs inside the sorted edge list
    bounds = np.zeros(NC * TPC * 2 + 1, np.int64)
    bounds[1:] = np.cumsum(cnt.reshape(-1))

    per_core = []
    for c in range(NC):
        idx_arr = np.zeros(TOT, np.int32)
        dl_arr = np.full(TOT, -1.0, np.float32)
        for t in range(TPC):
            p = 0
            for hh, spos, L in segs[t]:
                k = (c * TPC + t) * 2 + hh
                a, b = bounds[k], bounds[k + 1]
                n = b - a
                idx_arr[spos:spos + n] = s_src[a:b]
                dl_arr[spos:spos + n] = s_dl[a:b]
                p += L
        assert idx_arr.max() < 32768
        idx_w = np.tile(idx_arr.astype(np.int16).reshape(TOT // 16, 16).T, (8, 1)).copy()
        dl_w = dl_arr.reshape(TOT // 128, 128).T.astype(bf16).copy()

        degc = np.ones(NPAD, np.float32)
        degc[:NPC] = deg[c * NPC:(c + 1) * NPC]
        deg_pp = degc.reshape(TPC, 128).T.copy()
        deg_row = degc.reshape(1, NPAD).copy()

        xc = np.zeros((NPAD, x.shape[1]), np.float32)
        xc[:NPC] = x[c * NPC:(c + 1) * NPC]
        # tiled transposed x: block (t, k) = x[t-nodes, k-features].T, contiguous
        xtt = xc.reshape(TPC, 128, 3, 128).transpose(0, 2, 3, 1) \
                .reshape(TPC * 3 * 128, 128).astype(bf16)

        per_core.append(dict(idx=idx_w, dl=dl_w, deg_pp=deg_pp, deg_row=deg_row,
                             xtt=xtt))

    wshared = dict(
        W1=W1.astype(bf16), W2=W2.astype(bf16), W3=W3.astype(bf16),
        W4=W4.astype(bf16), Wl=Wl.astype(bf16),
        b1=b1.reshape(1, -1).astype(bf16), b2=b2.reshape(1, -1).astype(bf16),
        b3=b3.reshape(1, -1).astype(bf16), b4=b4.reshape(1, -1).astype(bf16),
        bl=bl.reshape(1, -1).astype(bf16),
    )
    cfg = dict(N=N, NPC=NPC, TPC=TPC, NPAD=NPAD, CH0=ch0, CH1=ch1, TOT=TOT,
               segs=segs, groups=groups, G=G)
    return cfg, per_core, wshared


def split_calls(pos, L, maxc):
    out = []
    while L > 0:
        c = min(L, maxc)
        out.append((pos, c))
        pos += c
        L -= c
    return out


def build_program(cfg, maxc128=1024, maxc256=1024, lrelu=True):
    N, NPC, TPC, NPAD = cfg["N"], cfg["NPC"], cfg["TPC"], cfg["NPAD"]
    CH0, CH1, TOT, segs = cfg["CH0"], cfg["CH1"], cfg["TOT"], cfg["segs"]

    nc = bacc.Bacc("TRN2", target_bir_lowering=False, debug=False,
                   num_devices=NC, num_swdge_queues=4)
    qctr = [0]

    def next_q():
        q = qctr[0] % 4
        qctr[0] += 1
        return q

    # ---- I/O ----
    xtt_t = nc.dram_tensor("xtt", [TPC * 3 * 128, 128], dt.bfloat16, kind="ExternalInput")
    idx_t = nc.dram_tensor("idx", [128, TOT // 16], dt.int16, kind="ExternalInput")
    dl_t = nc.dram_tensor("dl", [128, TOT // 128], dt.bfloat16, kind="ExternalInput")
    degpp_t = nc.dram_tensor("deg_pp", [128, TPC], dt.float32, kind="ExternalInput")
    degrow_t = nc.dram_tensor("deg_row", [1, NPAD], dt.float32, kind="ExternalInput")
    w_t = {k: nc.dram_tensor(k, list(s), dt.bfloat16, kind="ExternalInput")
           for k, s in dict(W1=(384, 128), W2=(128, 384), W3=(384, 256),
                            W4=(256, 384), Wl=(384, 128), b1=(1, 128),
                            b2=(1, 384), b3=(1, 256), b4=(1, 384),
                            bl=(1, 128)).items()}
    out_t = nc.dram_tensor("out", [NPC, 128], dt.float32, kind="ExternalOutput")

    # ---- internal DRAM: allgather shards + shared chunk tables ----
    FDIMS = [128, 128, 256, 256]     # payload width of agg pass 1..4
    ag_in = [nc.dram_tensor(f"agin{i}", [NPC, F], dt.bfloat16)
             for i, F in enumerate(FDIMS)]
    tabA = [nc.dram_tensor(f"tabA{i}", [NC * CH0, F], dt.bfloat16,
                           addr_space="Shared") for i, F in enumerate(FDIMS)]
    tabB = [nc.dram_tensor(f"tabB{i}", [NC * CH1, F], dt.bfloat16,
                           addr_space="Shared") for i, F in enumerate(FDIMS)]

    f32, bft = dt.float32, dt.bfloat16

    with tile.TileContext(nc) as tc:
        with tc.tile_pool(name="const", bufs=1) as cp:
            # ---------- constants / persistent ----------
            iota_i = cp.tile([128, 128], dt.int32)
            nc.gpsimd.iota(iota_i[:], pattern=[[1, 128]], base=0, channel_multiplier=0)
            iota_b = cp.tile([128, 128], bft)
            nc.vector.tensor_copy(iota_b[:], iota_i[:])
            ident_b = cp.tile([128, 128], bft)
            masks.make_identity(nc, ident_b[:])
            ones_row = cp.tile([1, 128], bft)
            nc.gpsimd.memset(ones_row[:], 1.0)

            idx_sb = cp.tile([128, TOT // 16], dt.int16)
            nc.sync.dma_start(out=idx_sb[:], in_=idx_t[:, :])
            dl_sb = cp.tile([128, TOT // 128], bft)
            nc.sync.dma_start(out=dl_sb[:], in_=dl_t[:, :])

            # weights into SBUF (k-chunks of 128 contraction rows)
            def wtiles(name, K, F):
                ts = []
                for k in range(K // 128):
                    w = cp.tile([128, F], bft, tag=f"{name}{k}")
                    nc.sync.dma_start(out=w[:], in_=w_t[name][k * 128:(k + 1) * 128, :])
                    ts.append(w)
                return ts
            W1sb = wtiles("W1", 384, 128)
            W2sb = wtiles("W2", 128, 384)
            W3sb = wtiles("W3", 384, 256)
            W4sb = wtiles("W4", 256, 384)
            Wlsb = wtiles("Wl", 384, 128)
            brow = {}
            for name, F in [("b1", 128), ("b2", 384), ("b3", 256), ("b4", 384), ("bl", 128)]:
                b = cp.tile([1, F], bft, tag=name)
                nc.sync.dma_start(out=b[:], in_=w_t[name][:, :])
                brow[name] = b

            # degree-derived scales
            deg_pp = cp.tile([128, TPC], f32)
            nc.sync.dma_start(out=deg_pp[:], in_=degpp_t[:, :])
            sq_pp = cp.tile([128, TPC], f32)
            nc.scalar.activation(sq_pp[:], deg_pp[:], mybir.ActivationFunctionType.Sqrt)
            dinv_pp = cp.tile([128, TPC], f32)
            nc.vector.reciprocal(dinv_pp[:], sq_pp[:])
            deginv_pp = cp.tile([128, TPC], f32)
            nc.vector.reciprocal(deginv_pp[:], deg_pp[:])
            deg_row = cp.tile([1, NPAD], f32)
            nc.sync.dma_start(out=deg_row[:], in_=degrow_t[:, :])
            sq_row = cp.tile([1, NPAD], bft)
            nc.scalar.activation(sq_row[:], deg_row[:], mybir.ActivationFunctionType.Sqrt)

            if lrelu:
                def act_leaky(out_ap, ps_ap, scale_tile, t, tmp_pool):
                    nc.scalar.activation(out_ap, ps_ap,
                                         mybir.ActivationFunctionType.Lrelu,
                                         bias=0.0, scale=scale_tile[:, t:t + 1],
                                         alpha=0.01)
            else:
                # leaky(s*x) = relu(0.99*s*x) + 0.01*s*x  (sim lacks Lrelu)
                s99 = {}
                s001 = {}
                for nm, tl in (("dinv", dinv_pp), ("deginv", deginv_pp)):
                    a = cp.tile([128, TPC], f32, tag=f"{nm}99")
                    nc.vector.tensor_scalar_mul(a[:], tl[:], 0.99)
                    b = cp.tile([128, TPC], f32, tag=f"{nm}001")
                    nc.vector.tensor_scalar_mul(b[:], tl[:], 0.01)
                    s99[id(tl)] = a
                    s001[id(tl)] = b

                def act_leaky(out_ap, ps_ap, scale_tile, t, tmp_pool):
                    r = tmp_pool.tile([128, out_ap.shape[-1]], f32, tag="lrl_r")
                    nc.scalar.activation(r[:], ps_ap,
                                         mybir.ActivationFunctionType.Relu,
                                         bias=0.0,
                                         scale=s99[id(scale_tile)][:, t:t + 1])
                    t1 = tmp_pool.tile([128, out_ap.shape[-1]], f32, tag="lrl_t")
                    nc.vector.tensor_scalar(t1[:], ps_ap,
                                            s001[id(scale_tile)][:, t:t + 1], None,
                                            mybir.AluOpType.mult)
                    nc.vector.tensor_add(out_ap, r[:], t1[:])

            nv = lambda t: min(128, NPC - t * 128)        # valid rows of tile t


            def allgather(i, ck):
                if ck == 0:
                    in_ap = ag_in[i][0:CH0, :]
                    out_ap = tabA[i].ap()
                else:
                    in_ap = ag_in[i][CH0:NPC, :]
                    out_ap = tabB[i].ap()
                nc.gpsimd.collective_compute(
                    "AllGather", mybir.AluOpType.bypass,
                    replica_groups=[list(range(NC))],
                    ins=[in_ap.opt()], outs=[out_ap.opt()])

            allgather0_hook = lambda: allgather(0, 0)

            # ---------- phase B: dense1 -> T1 ----------
            with tc.tile_pool(name="xp", bufs=6) as xp, \
                 tc.tile_pool(name="t1p", bufs=4) as t1p, \
                 tc.tile_pool(name="psB", bufs=4, space="PSUM") as psB:
                for t in range(TPC):
                    xts = []
                    for k in range(3):
                        xt = xp.tile([128, 128], bft, tag="xt")
                        r0 = (t * 3 + k) * 128
                        nc.sync.dma_start(out=xt[:], in_=xtt_t[r0:r0 + 128, :])
                        xts.append(xt)
                    ps = psB.tile([128, 128], f32, tag="ps1")
                    for k in range(3):
                        nc.tensor.matmul(ps[:], lhsT=xts[k][:], rhs=W1sb[k][:],
                                         start=(k == 0), stop=(k == 2))
                    T1t = t1p.tile([128, 128], bft, tag="t1")
                    nc.vector.tensor_scalar(T1t[:], ps[:], dinv_pp[:, t:t + 1], None,
                                            mybir.AluOpType.mult)
                    nc.sync.dma_start(out=ag_in[0][t * 128:t * 128 + nv(t), :],
                                      in_=T1t[:nv(t), :])
                    if t == 24:
                        allgather0_hook()

            allgather(0, 1)

            # ---------- generic aggregation pass (P-stationary) ----------
            def agg_pass(pi, F, post, binit_bias=None, after_tile=None):
                maxc = maxc128 if F == 128 else maxc256
                aggbufs = {0: 6, 1: 4, 2: 4, 3: 2}[pi]
                half0 = tabA[pi][:, :]
                half1 = tabB[pi][:, :]
                after_tile = after_tile or {}
                with tc.tile_pool(name=f"g{pi}", bufs=14) as gp, \
                     tc.tile_pool(name=f"pp{pi}", bufs=14) as pp, \
                     tc.tile_pool(name=f"sl{pi}", bufs=4) as slp, \
                     tc.tile_pool(name=f"agg{pi}", bufs=aggbufs, space="PSUM") as ap_:
                    D = {0: 1, 1: 1, 2: 1, 3: 1}[pi]
                    groups = cfg["groups"]
                    ng = len(groups)
                    st = {}

                    def open_group(gi):
                        gmeta = groups[gi]
                        tiles = sorted({t for hh in (0, 1)
                                        for t, _, _ in gmeta[hh]})
                        nt = len(tiles)
                        # one PSUM bank-tile holds the whole group's agg
                        # slices; all matmuls into it form ONE accumulation
                        # group (per-slice start=True would clear the bank).
                        bank = ap_.tile([128, nt * F], f32, tag="agg",
                                        name="aggbank")
                        pst = {}
                        left = {t: sum(L for _, _, L in segs[t]) // 128
                                for t in tiles}
                        nmm = sum(left.values()) + \
                            (nt if binit_bias is not None else 0)
                        # bank-wide self-loop opener: + own table rows
                        slg = slp.tile([128, nt * F], bft, tag="sl",
                                       name="slg")
                        for j, t in enumerate(tiles):
                            pst[t] = bank[:, j * F:(j + 1) * F]
                            r1 = min((t + 1) * 128, NPC)
                            if r1 - t * 128 < 128:
                                nc.vector.memzero(slg[:, j * F:(j + 1) * F])
                            nc.scalar.dma_start(
                                out=slg[:r1 - t * 128, j * F:(j + 1) * F],
                                in_=ag_in[pi][t * 128:r1, :])
                        for c0 in range(0, nt * F, 512):
                            cw = min(512, nt * F - c0)
                            nc.tensor.matmul(bank[:, c0:c0 + cw],
                                             lhsT=ident_b[:],
                                             rhs=slg[:, c0:c0 + cw],
                                             start=True, stop=False,
                                             skip_group_check=True)
                        gs = dict(tiles=tiles, pst=pst, left=left,
                                  nmm=[nmm])
                        if binit_bias is not None:
                            for j, t in enumerate(tiles):
                                gs["nmm"][0] -= 1
                                nc.tensor.matmul(
                                    pst[t],
                                    lhsT=sq_row[0:1, t * 128:(t + 1) * 128],
                                    rhs=binit_bias[:],
                                    start=False, stop=(gs["nmm"][0] == 0))
                        return gs

                    def emit_chunk(gi, hh):
                        gs = st[gi]
                        gmeta = groups[gi]
                        src_ap = half0 if hh == 0 else half1
                        spans = gmeta[hh]
                        if not spans:
                            return
                        gpos = spans[0][1]
                        gend = spans[-1][1] + spans[-1][2]
                        for cpos, clen in split_calls(gpos, gend - gpos, maxc):
                            nch = clen // 128
                            g = gp.tile([128, nch * F], bft, tag="g",
                                        name="g")
                            g3 = g[:].rearrange("p (c e) -> p c e", e=F)
                            nc.gpsimd.dma_gather(
                                out_ap=g3, in_ap=src_ap,
                                idxs_ap=idx_sb[:, cpos // 16:(cpos + clen) // 16],
                                num_idxs=clen, num_idxs_reg=clen, elem_size=F,
                                single_packet=False, queue_num=next_q())
                            P = pp.tile([128, clen], bft, tag="P", name="P")
                            P3 = P[:].rearrange("p (c d) -> p c d", d=128)
                            nc.vector.tensor_tensor(
                                P3,
                                iota_b[:].unsqueeze(1).broadcast_to([128, nch, 128]),
                                dl_sb[:, cpos // 128:(cpos + clen) // 128]
                                    .unsqueeze(2).broadcast_to([128, nch, 128]),
                                mybir.AluOpType.is_equal)
                            for j in range(nch):
                                epos = cpos + j * 128
                                t = next(tt for tt, p0, L in spans
                                         if p0 <= epos < p0 + L)
                                gs["left"][t] -= 1
                                gs["nmm"][0] -= 1
                                nc.tensor.matmul(
                                    gs["pst"][t],
                                    lhsT=P[:, j * 128:(j + 1) * 128],
                                    rhs=g[:, j * F:(j + 1) * F],
                                    start=False,
                                    stop=(gs["nmm"][0] == 0))

                    def close_group(gi):
                        gs = st.pop(gi)
                        for t in gs["tiles"]:
                            post(t, gs["pst"][t])
                            if t in after_tile:
                                after_tile[t]()

                    # software-pipelined emission: group gi's chunk-1 spans
                    # are emitted D groups later so chunk-0 gathers of later
                    # groups fill the wait for the chunk-1 all-gather.
                    for gi in range(ng + D):
                        if gi < ng:
                            st[gi] = open_group(gi)
                            emit_chunk(gi, 0)
                        if gi - D >= 0:
                            emit_chunk(gi - D, 1)
                            close_group(gi - D)

            # ---------- pass C: agg1 -> T2 ----------
            with tc.tile_pool(name="t2p", bufs=4) as t2p:
                def post_c(t, ps):
                    T2t = t2p.tile([128, 128], bft, tag="t2")
                    act_leaky(T2t[:], ps, deginv_pp, t, t2p)
                    nc.sync.dma_start(out=ag_in[1][t * 128:t * 128 + nv(t), :],
                                      in_=T2t[:nv(t), :])
                agg_pass(0, 128, post_c, binit_bias=brow["b1"],
                         after_tile={24: lambda: allgather(1, 0),
                                     TPC - 1: lambda: allgather(1, 1)})

            # ---------- pass D1 (+fused dense2+dense3): agg2 -> T3 ----------
            with tc.tile_pool(name="hp", bufs=6) as hp, \
                 tc.tile_pool(name="t3p", bufs=4) as t3p, \
                 tc.tile_pool(name="psD", bufs=1, space="PSUM") as psD, \
                 tc.tile_pool(name="trD", bufs=1, space="PSUM") as trD:
                def post_d1(t, ps):
                    s2 = hp.tile([128, 128], bft, tag="s2", name="s2")
                    nc.vector.tensor_copy(s2[:], ps)
                    # transpose S2 tile -> lhsT for dense2
                    trs = trD.tile([128, 128], bft, tag="trs")
                    nc.tensor.matmul(trs[:], lhsT=s2[:],
                                     rhs=ident_b[:], is_transpose=True)
                    s2t = hp.tile([128, 128], bft, tag="s2t")
                    nc.vector.tensor_copy(s2t[:], trs[:])
                    ps2 = psD.tile([128, 384], f32, tag="ps2")
                    nc.tensor.matmul(ps2[:], lhsT=sq_row[0:1, t * 128:(t + 1) * 128],
                                     rhs=brow["b2"][:], start=True, stop=False)
                    nc.tensor.matmul(ps2[:], lhsT=s2t[:],
                                     rhs=W2sb[0][:], start=False, stop=True)
                    h2 = hp.tile([128, 384], bft, tag="h2")
                    act_leaky(h2[:], ps2[:], dinv_pp, t, hp)
                    trp = trD.tile([128, 384], bft, tag="tr")
                    for k in range(3):
                        nc.tensor.matmul(trp[:, k * 128:(k + 1) * 128],
                                         lhsT=h2[:, k * 128:(k + 1) * 128],
                                         rhs=ident_b[:], is_transpose=True)
                    h2t = hp.tile([128, 384], bft, tag="h2t")
                    nc.vector.tensor_copy(h2t[:], trp[:])
                    ps3 = psD.tile([128, 256], f32, tag="ps3")
                    for k in range(3):
                        nc.tensor.matmul(ps3[:], lhsT=h2t[:, k * 128:(k + 1) * 128],
                                         rhs=W3sb[k][:], start=(k == 0), stop=(k == 2))
                    T3t = t3p.tile([128, 256], bft, tag="t3")
                    nc.vector.tensor_scalar(T3t[:], ps3[:], dinv_pp[:, t:t + 1], None,
                                            mybir.AluOpType.mult)
                    nc.sync.dma_start(out=ag_in[2][t * 128:t * 128 + nv(t), :],
                                      in_=T3t[:nv(t), :])
                agg_pass(1, 128, post_d1,
                         after_tile={24: lambda: allgather(2, 0),
                                     TPC - 1: lambda: allgather(2, 1)})

            # ---------- pass E: agg3 -> T4 ----------
            with tc.tile_pool(name="t4p", bufs=4) as t4p:
                def post_e(t, ps):
                    T4t = t4p.tile([128, 256], bft, tag="t4")
                    act_leaky(T4t[:], ps, deginv_pp, t, t4p)
                    nc.sync.dma_start(out=ag_in[3][t * 128:t * 128 + nv(t), :],
                                      in_=T4t[:nv(t), :])
                agg_pass(2, 256, post_e, binit_bias=brow["b3"],
                         after_tile={24: lambda: allgather(3, 0),
                                     TPC - 1: lambda: allgather(3, 1)})

            # ---------- pass F1 (+fused dense4 + dense5) -> out ----------
            with tc.tile_pool(name="hp4", bufs=6) as hp4, \
                 tc.tile_pool(name="op", bufs=4) as op, \
                 tc.tile_pool(name="psF", bufs=1, space="PSUM") as psF, \
                 tc.tile_pool(name="trF", bufs=1, space="PSUM") as trF:
                def post_f1(t, ps):
                    s4 = hp4.tile([128, 256], bft, tag="s4", name="s4")
                    nc.vector.tensor_copy(s4[:], ps)
                    # transpose S4 tile -> 2 lhsT chunks for dense4
                    s4t = hp4.tile([128, 256], bft, tag="s4t")
                    for fk in range(2):
                        trs = trF.tile([128, 128], bft, tag="trs4")
                        nc.tensor.matmul(
                            trs[:],
                            lhsT=s4[:, fk * 128:(fk + 1) * 128],
                            rhs=ident_b[:], is_transpose=True)
                        nc.vector.tensor_copy(s4t[:, fk * 128:(fk + 1) * 128], trs[:])
                    ps4 = psF.tile([128, 384], f32, tag="ps4")
                    nc.tensor.matmul(ps4[:], lhsT=sq_row[0:1, t * 128:(t + 1) * 128],
                                     rhs=brow["b4"][:], start=True, stop=False)
                    for fk in range(2):
                        nc.tensor.matmul(ps4[:],
                                         lhsT=s4t[:, fk * 128:(fk + 1) * 128],
                                         rhs=W4sb[fk][:], start=False, stop=(fk == 1))
                    h4 = hp4.tile([128, 384], bft, tag="h4")
                    act_leaky(h4[:], ps4[:], dinv_pp, t, hp4)
                    trp = trF.tile([128, 384], bft, tag="tr4")
                    for k in range(3):
                        nc.tensor.matmul(trp[:, k * 128:(k + 1) * 128],
                                         lhsT=h4[:, k * 128:(k + 1) * 128],
                                         rhs=ident_b[:], is_transpose=True)
                    h4t = hp4.tile([128, 384], bft, tag="h4t")
                    nc.vector.tensor_copy(h4t[:], trp[:])
                    ps5 = psF.tile([128, 128], f32, tag="ps5")
                    nc.tensor.matmul(ps5[:], lhsT=ones_row[:], rhs=brow["bl"][:],
                                     start=True, stop=False)
                    for k in range(3):
                        nc.tensor.matmul(ps5[:], lhsT=h4t[:, k * 128:(k + 1) * 128],
                                         rhs=Wlsb[k][:], start=False, stop=(k == 2))
                    ot = op.tile([128, 128], f32, tag="o")
                    nc.scalar.activation(ot[:], ps5[:], mybir.ActivationFunctionType.Relu)
                    nc.sync.dma_start(out=out_t[t * 128:t * 128 + nv(t), :],
                                      in_=ot[:nv(t), :])
                agg_pass(3, 256, post_f1)

    nc.compile()
    return nc


def kernel(x, edge_index, W1, b1, W2, b2, W3, b3, W4, b4, Wl, bl,
           trace=False):
    x = np.asarray(x, dtype=np.float32)
    edge_index = np.asarray(edge_index)
    cfg, per_core, wshared = build_host_data(
        x, edge_index, W1, b1, W2, b2, W3, b3, W4, b4, Wl, bl)
    nc = build_program(cfg)
    in_maps = []
    for c in range(NC):
        d = dict(per_core[c])
        m = {"xtt": d["xtt"], "idx": d["idx"], "dl": d["dl"],
             "deg_pp": d["deg_pp"], "deg_row": d["deg_row"]}
        m.update(wshared)
        in_maps.append(m)
    res = run_bass_kernel_spmd(nc, in_maps, core_ids=list(range(NC)),
                               trace=trace)
    out = np.concatenate([res.results[c]["out"] for c in range(NC)], axis=0)
    kernel.last_exec_time_ns = res.exec_time_ns
    kernel.last_results = res
    return out



# revision 41
# speedup vs baseline: 1.0268x; 1.0268x over previous
import sys, os
sys.path.insert(0, "/opt/trn_rl_repo")
os.environ.setdefault("NEURON_RT_LOG_LEVEL", "WARNING")
import numpy as np
import ml_dtypes

import concourse.bass as bass
import concourse.bacc as bacc
import concourse.mybir as mybir
import concourse.tile as tile
from concourse import masks
from concourse.bass_utils import run_bass_kernel_spmd

dt = mybir.dt
bf16 = ml_dtypes.bfloat16
NC = 8


def ceil128(v):
    return (int(v) + 127) // 128 * 128


def build_host_data(x, edge_index, W1, b1, W2, b2, W3, b3, W4, b4, Wl, bl,
                    ch0=3200):
    """Partition the graph by destination node across 8 cores and build all
    per-core device input arrays plus the (core-uniform) segment layout.

    Sources are split into two chunks by their offset within the OWNER
    core's shard (off < ch0 vs off >= ch0).  Each chunk has its own
    rank-major all-gathered table, so the chunk-0 collective can fire as
    soon as the producing pass finishes its first 25 destination tiles."""
    N = x.shape[0]
    NPC = N // NC
    TPC = (NPC + 127) // 128
    NPAD = TPC * 128
    ch1 = NPC - ch0

    # self-loops are applied on-device via an identity matmul over the
    # core-local table shard; only real edges go through the gather path.
    src = edge_index[0].astype(np.int64)
    dst = edge_index[1].astype(np.int64)
    deg = (np.bincount(dst, minlength=N) + 1).astype(np.float32)

    core = dst // NPC
    dstl = dst - core * NPC
    tl = dstl >> 7
    dl128 = (dstl & 127).astype(np.float32)
    sowner = src // NPC
    soff = src - sowner * NPC
    h = (soff >= ch0).astype(np.int64)
    # index of the source row inside its chunk table (rank-major)
    cidx = np.where(h == 0, sowner * ch0 + soff,
                    sowner * ch1 + (soff - ch0))

    order = np.lexsort((cidx, h, tl, core))
    s_src = cidx[order]
    s_dl = dl128[order]

    key = (core * TPC + tl) * 2 + h
    cnt = np.bincount(key, minlength=NC * TPC * 2).reshape(NC, TPC, 2)
    m = np.maximum(cnt.max(axis=0), 0)
    m = ((m + 127) // 128 * 128).astype(np.int64)          # [TPC, 2]

    # segment table (uniform across cores). Stream layout is GROUP-major:
    # tiles are grouped G at a time; within a group all h=0 segments come
    # first (one gather base), then all h=1 segments. segs[t] keeps the
    # per-tile (h, pos, len) list for data placement.
    G = 4
    segs = [[] for _ in range(TPC)]
    groups = []          # list of (h -> list of (t, pos, len))
    pos = 0
    for g0 in range(0, TPC, G):
        tl = list(range(g0, min(g0 + G, TPC)))
        gmeta = {0: [], 1: []}
        for hh in (0, 1):
            for t in tl:
                L = int(m[t, hh])
                if L:
                    gmeta[hh].append((t, pos, L))
                    segs[t].append((hh, pos, L))
                    pos += L
        groups.append(gmeta)
    TOT = pos
    assert TOT % 128 == 0

    # boundaries of (core, tile, half) runs inside the sorted edge list
    bounds = np.zeros(NC * TPC * 2 + 1, np.int64)
    bounds[1:] = np.cumsum(cnt.reshape(-1))

    per_core = []
    for c in range(NC):
        idx_arr = np.zeros(TOT, np.int32)
        dl_arr = np.full(TOT, -1.0, np.float32)
        for t in range(TPC):
            p = 0
            for hh, spos, L in segs[t]:
                k = (c * TPC + t) * 2 + hh
                a, b = bounds[k], bounds[k + 1]
                n = b - a
                idx_arr[spos:spos + n] = s_src[a:b]
                dl_arr[spos:spos + n] = s_dl[a:b]
                p += L
        assert idx_arr.max() < 32768
        idx_w = np.tile(idx_arr.astype(np.int16).reshape(TOT // 16, 16).T, (8, 1)).copy()
        dl_w = dl_arr.reshape(TOT // 128, 128).T.astype(bf16).copy()

        degc = np.ones(NPAD, np.float32)
        degc[:NPC] = deg[c * NPC:(c + 1) * NPC]
        deg_pp = degc.reshape(TPC, 128).T.copy()
        deg_row = degc.reshape(1, NPAD).copy()

        xc = np.zeros((NPAD, x.shape[1]), np.float32)
        xc[:NPC] = x[c * NPC:(c + 1) * NPC]
        # tiled transposed x: block (t, k) = x[t-nodes, k-features].T, contiguous
        xtt = xc.reshape(TPC, 128, 3, 128).transpose(0, 2, 3, 1) \
                .reshape(TPC * 3 * 128, 128).astype(bf16)

        per_core.append(dict(idx=idx_w, dl=dl_w, deg_pp=deg_pp, deg_row=deg_row,
                             xtt=xtt))

    wshared = dict(
        W1=W1.astype(bf16), W2=W2.astype(bf16), W3=W3.astype(bf16),
        W4=W4.astype(bf16), Wl=Wl.astype(bf16),
        b1=b1.reshape(1, -1).astype(bf16), b2=b2.reshape(1, -1).astype(bf16),
        b3=b3.reshape(1, -1).astype(bf16), b4=b4.reshape(1, -1).astype(bf16),
        bl=bl.reshape(1, -1).astype(bf16),
    )
    cfg = dict(N=N, NPC=NPC, TPC=TPC, NPAD=NPAD, CH0=ch0, CH1=ch1, TOT=TOT,
               segs=segs, groups=groups, G=G)
    return cfg, per_core, wshared


def split_calls(pos, L, maxc):
    out = []
    while L > 0:
        c = min(L, maxc)
        out.append((pos, c))
        pos += c
        L -= c
    return out


def build_program(cfg, maxc128=1024, maxc256=1024, lrelu=True):
    N, NPC, TPC, NPAD = cfg["N"], cfg["NPC"], cfg["TPC"], cfg["NPAD"]
    CH0, CH1, TOT, segs = cfg["CH0"], cfg["CH1"], cfg["TOT"], cfg["segs"]

    nc = bacc.Bacc("TRN2", target_bir_lowering=False, debug=False,
                   num_devices=NC, num_swdge_queues=4)
    qctr = [0]

    def next_q():
        q = qctr[0] % 4
        qctr[0] += 1
        return q

    # ---- I/O ----
    xtt_t = nc.dram_tensor("xtt", [TPC * 3 * 128, 128], dt.bfloat16, kind="ExternalInput")
    idx_t = nc.dram_tensor("idx", [128, TOT // 16], dt.int16, kind="ExternalInput")
    dl_t = nc.dram_tensor("dl", [128, TOT // 128], dt.bfloat16, kind="ExternalInput")
    degpp_t = nc.dram_tensor("deg_pp", [128, TPC], dt.float32, kind="ExternalInput")
    degrow_t = nc.dram_tensor("deg_row", [1, NPAD], dt.float32, kind="ExternalInput")
    w_t = {k: nc.dram_tensor(k, list(s), dt.bfloat16, kind="ExternalInput")
           for k, s in dict(W1=(384, 128), W2=(128, 384), W3=(384, 256),
                            W4=(256, 384), Wl=(384, 128), b1=(1, 128),
                            b2=(1, 384), b3=(1, 256), b4=(1, 384),
                            bl=(1, 128)).items()}
    out_t = nc.dram_tensor("out", [NPC, 128], dt.float32, kind="ExternalOutput")

    # ---- internal DRAM: allgather shards + shared chunk tables ----
    FDIMS = [128, 128, 256, 256]     # payload width of agg pass 1..4
    ag_in = [nc.dram_tensor(f"agin{i}", [NPC, F], dt.bfloat16)
             for i, F in enumerate(FDIMS)]
    tabA = [nc.dram_tensor(f"tabA{i}", [NC * CH0, F], dt.bfloat16,
                           addr_space="Shared") for i, F in enumerate(FDIMS)]
    tabB = [nc.dram_tensor(f"tabB{i}", [NC * CH1, F], dt.bfloat16,
                           addr_space="Shared") for i, F in enumerate(FDIMS)]

    f32, bft = dt.float32, dt.bfloat16

    with tile.TileContext(nc) as tc:
        with tc.tile_pool(name="const", bufs=1) as cp:
            # ---------- constants / persistent ----------
            iota_i = cp.tile([128, 128], dt.int32)
            nc.gpsimd.iota(iota_i[:], pattern=[[1, 128]], base=0, channel_multiplier=0)
            iota_b = cp.tile([128, 128], bft)
            nc.vector.tensor_copy(iota_b[:], iota_i[:])
            ident_b = cp.tile([128, 128], bft)
            masks.make_identity(nc, ident_b[:])
            ones_row = cp.tile([1, 128], bft)
            nc.gpsimd.memset(ones_row[:], 1.0)

            idx_sb = cp.tile([128, TOT // 16], dt.int16)
            nc.sync.dma_start(out=idx_sb[:], in_=idx_t[:, :])
            dl_sb = cp.tile([128, TOT // 128], bft)
            nc.sync.dma_start(out=dl_sb[:], in_=dl_t[:, :])

            # weights into SBUF (k-chunks of 128 contraction rows)
            def wtiles(name, K, F):
                ts = []
                for k in range(K // 128):
                    w = cp.tile([128, F], bft, tag=f"{name}{k}")
                    nc.sync.dma_start(out=w[:], in_=w_t[name][k * 128:(k + 1) * 128, :])
                    ts.append(w)
                return ts
            W1sb = wtiles("W1", 384, 128)
            W2sb = wtiles("W2", 128, 384)
            W3sb = wtiles("W3", 384, 256)
            W4sb = wtiles("W4", 256, 384)
            Wlsb = wtiles("Wl", 384, 128)
            brow = {}
            for name, F in [("b1", 128), ("b2", 384), ("b3", 256), ("b4", 384), ("bl", 128)]:
                b = cp.tile([1, F], bft, tag=name)
                nc.sync.dma_start(out=b[:], in_=w_t[name][:, :])
                brow[name] = b

            # degree-derived scales
            deg_pp = cp.tile([128, TPC], f32)
            nc.sync.dma_start(out=deg_pp[:], in_=degpp_t[:, :])
            sq_pp = cp.tile([128, TPC], f32)
            nc.scalar.activation(sq_pp[:], deg_pp[:], mybir.ActivationFunctionType.Sqrt)
            dinv_pp = cp.tile([128, TPC], f32)
            nc.vector.reciprocal(dinv_pp[:], sq_pp[:])
            deginv_pp = cp.tile([128, TPC], f32)
            nc.vector.reciprocal(deginv_pp[:], deg_pp[:])
            deg_row = cp.tile([1, NPAD], f32)
            nc.sync.dma_start(out=deg_row[:], in_=degrow_t[:, :])
            sq_row = cp.tile([1, NPAD], bft)
            nc.scalar.activation(sq_row[:], deg_row[:], mybir.ActivationFunctionType.Sqrt)

            if lrelu:
                def act_leaky(out_ap, ps_ap, scale_tile, t, tmp_pool):
                    nc.scalar.activation(out_ap, ps_ap,
                                         mybir.ActivationFunctionType.Lrelu,
                                         bias=0.0, scale=scale_tile[:, t:t + 1],
                                         alpha=0.01)
            else:
                # leaky(s*x) = relu(0.99*s*x) + 0.01*s*x  (sim lacks Lrelu)
                s99 = {}
                s001 = {}
                for nm, tl in (("dinv", dinv_pp), ("deginv", deginv_pp)):
                    a = cp.tile([128, TPC], f32, tag=f"{nm}99")
                    nc.vector.tensor_scalar_mul(a[:], tl[:], 0.99)
                    b = cp.tile([128, TPC], f32, tag=f"{nm}001")
                    nc.vector.tensor_scalar_mul(b[:], tl[:], 0.01)
                    s99[id(tl)] = a
                    s001[id(tl)] = b

                def act_leaky(out_ap, ps_ap, scale_tile, t, tmp_pool):
                    r = tmp_pool.tile([128, out_ap.shape[-1]], f32, tag="lrl_r")
                    nc.scalar.activation(r[:], ps_ap,
                                         mybir.ActivationFunctionType.Relu,
                                         bias=0.0,
                                         scale=s99[id(scale_tile)][:, t:t + 1])
                    t1 = tmp_pool.tile([128, out_ap.shape[-1]], f32, tag="lrl_t")
                    nc.vector.tensor_scalar(t1[:], ps_ap,
                                            s001[id(scale_tile)][:, t:t + 1], None,
                                            mybir.AluOpType.mult)
                    nc.vector.tensor_add(out_ap, r[:], t1[:])

            nv = lambda t: min(128, NPC - t * 128)        # valid rows of tile t


            def allgather(i, ck):
                if ck == 0:
                    in_ap = ag_in[i][0:CH0, :]
                    out_ap = tabA[i].ap()
                else:
                    in_ap = ag_in[i][CH0:NPC, :]
                    out_ap = tabB[i].ap()
                nc.gpsimd.collective_compute(
                    "AllGather", mybir.AluOpType.bypass,
                    replica_groups=[list(range(NC))],
                    ins=[in_ap.opt()], outs=[out_ap.opt()])

            allgather0_hook = lambda: allgather(0, 0)

            # ---------- phase B: dense1 -> T1 ----------
            with tc.tile_pool(name="xp", bufs=6) as xp, \
                 tc.tile_pool(name="t1p", bufs=4) as t1p, \
                 tc.tile_pool(name="psB", bufs=4, space="PSUM") as psB:
                for t in range(TPC):
                    xts = []
                    for k in range(3):
                        xt = xp.tile([128, 128], bft, tag="xt")
                        r0 = (t * 3 + k) * 128
                        nc.scalar.dma_start(out=xt[:], in_=xtt_t[r0:r0 + 128, :])
                        xts.append(xt)
                    ps = psB.tile([128, 128], f32, tag="ps1")
                    for k in range(3):
                        nc.tensor.matmul(ps[:], lhsT=xts[k][:], rhs=W1sb[k][:],
                                         start=(k == 0), stop=(k == 2))
                    T1t = t1p.tile([128, 128], bft, tag="t1")
                    nc.vector.tensor_scalar(T1t[:], ps[:], dinv_pp[:, t:t + 1], None,
                                            mybir.AluOpType.mult)
                    nc.sync.dma_start(out=ag_in[0][t * 128:t * 128 + nv(t), :],
                                      in_=T1t[:nv(t), :])
                    if t == 24:
                        allgather0_hook()

            allgather(0, 1)

            # ---------- generic aggregation pass (P-stationary) ----------
            def agg_pass(pi, F, post, binit_bias=None, after_tile=None):
                maxc = maxc128 if F == 128 else maxc256
                aggbufs = {0: 6, 1: 4, 2: 4, 3: 2}[pi]
                half0 = tabA[pi][:, :]
                half1 = tabB[pi][:, :]
                after_tile = after_tile or {}
                with tc.tile_pool(name=f"g{pi}", bufs=14) as gp, \
                     tc.tile_pool(name=f"pp{pi}", bufs=14) as pp, \
                     tc.tile_pool(name=f"sl{pi}", bufs=4) as slp, \
                     tc.tile_pool(name=f"agg{pi}", bufs=aggbufs, space="PSUM") as ap_:
                    D = {0: 1, 1: 1, 2: 1, 3: 1}[pi]
                    groups = cfg["groups"]
                    ng = len(groups)
                    st = {}

                    def open_group(gi):
                        gmeta = groups[gi]
                        tiles = sorted({t for hh in (0, 1)
                                        for t, _, _ in gmeta[hh]})
                        nt = len(tiles)
                        # one PSUM bank-tile holds the whole group's agg
                        # slices; all matmuls into it form ONE accumulation
                        # group (per-slice start=True would clear the bank).
                        bank = ap_.tile([128, nt * F], f32, tag="agg",
                                        name="aggbank")
                        pst = {}
                        left = {t: sum(L for _, _, L in segs[t]) // 128
                                for t in tiles}
                        nmm = sum(left.values()) + \
                            (nt if binit_bias is not None else 0)
                        # bank-wide self-loop opener: + own table rows
                        slg = slp.tile([128, nt * F], bft, tag="sl",
                                       name="slg")
                        for j, t in enumerate(tiles):
                            pst[t] = bank[:, j * F:(j + 1) * F]
                            r1 = min((t + 1) * 128, NPC)
                            if r1 - t * 128 < 128:
                                nc.vector.memzero(slg[:, j * F:(j + 1) * F])
                            nc.scalar.dma_start(
                                out=slg[:r1 - t * 128, j * F:(j + 1) * F],
                                in_=ag_in[pi][t * 128:r1, :])
                        for c0 in range(0, nt * F, 512):
                            cw = min(512, nt * F - c0)
                            nc.tensor.matmul(bank[:, c0:c0 + cw],
                                             lhsT=ident_b[:],
                                             rhs=slg[:, c0:c0 + cw],
                                             start=True, stop=False,
                                             skip_group_check=True)
                        gs = dict(tiles=tiles, pst=pst, left=left,
                                  nmm=[nmm])
                        if binit_bias is not None:
                            for j, t in enumerate(tiles):
                                gs["nmm"][0] -= 1
                                nc.tensor.matmul(
                                    pst[t],
                                    lhsT=sq_row[0:1, t * 128:(t + 1) * 128],
                                    rhs=binit_bias[:],
                                    start=False, stop=(gs["nmm"][0] == 0))
                        return gs

                    def emit_chunk(gi, hh):
                        gs = st[gi]
                        gmeta = groups[gi]
                        src_ap = half0 if hh == 0 else half1
                        spans = gmeta[hh]
                        if not spans:
                            return
                        gpos = spans[0][1]
                        gend = spans[-1][1] + spans[-1][2]
                        for cpos, clen in split_calls(gpos, gend - gpos, maxc):
                            nch = clen // 128
                            g = gp.tile([128, nch * F], bft, tag="g",
                                        name="g")
                            g3 = g[:].rearrange("p (c e) -> p c e", e=F)
                            nc.gpsimd.dma_gather(
                                out_ap=g3, in_ap=src_ap,
                                idxs_ap=idx_sb[:, cpos // 16:(cpos + clen) // 16],
                                num_idxs=clen, num_idxs_reg=clen, elem_size=F,
                                single_packet=False, queue_num=next_q())
                            P = pp.tile([128, clen], bft, tag="P", name="P")
                            P3 = P[:].rearrange("p (c d) -> p c d", d=128)
                            nc.vector.tensor_tensor(
                                P3,
                                iota_b[:].unsqueeze(1).broadcast_to([128, nch, 128]),
                                dl_sb[:, cpos // 128:(cpos + clen) // 128]
                                    .unsqueeze(2).broadcast_to([128, nch, 128]),
                                mybir.AluOpType.is_equal)
                            for j in range(nch):
                                epos = cpos + j * 128
                                t = next(tt for tt, p0, L in spans
                                         if p0 <= epos < p0 + L)
                                gs["left"][t] -= 1
                                gs["nmm"][0] -= 1
                                nc.tensor.matmul(
                                    gs["pst"][t],
                                    lhsT=P[:, j * 128:(j + 1) * 128],
                                    rhs=g[:, j * F:(j + 1) * F],
                                    start=False,
                                    stop=(gs["nmm"][0] == 0))

                    def close_group(gi):
                        gs = st.pop(gi)
                        for t in gs["tiles"]:
                            post(t, gs["pst"][t])
                            if t in after_tile:
                                after_tile[t]()

                    # software-pipelined emission: group gi's chunk-1 spans
                    # are emitted D groups later so chunk-0 gathers of later
                    # groups fill the wait for the chunk-1 all-gather.
                    for gi in range(ng + D):
                        if gi < ng:
                            st[gi] = open_group(gi)
                            emit_chunk(gi, 0)
                        if gi - D >= 0:
                            emit_chunk(gi - D, 1)
                            close_group(gi - D)

            # ---------- pass C: agg1 -> T2 ----------
            with tc.tile_pool(name="t2p", bufs=4) as t2p:
                def post_c(t, ps):
                    T2t = t2p.tile([128, 128], bft, tag="t2")
                    act_leaky(T2t[:], ps, deginv_pp, t, t2p)
                    nc.sync.dma_start(out=ag_in[1][t * 128:t * 128 + nv(t), :],
                                      in_=T2t[:nv(t), :])
                agg_pass(0, 128, post_c, binit_bias=brow["b1"],
                         after_tile={24: lambda: allgather(1, 0),
                                     TPC - 1: lambda: allgather(1, 1)})

            # ---------- pass D1 (+fused dense2+dense3): agg2 -> T3 ----------
            with tc.tile_pool(name="hp", bufs=6) as hp, \
                 tc.tile_pool(name="t3p", bufs=4) as t3p, \
                 tc.tile_pool(name="psD", bufs=1, space="PSUM") as psD, \
                 tc.tile_pool(name="trD", bufs=1, space="PSUM") as trD:
                def post_d1(t, ps):
                    s2 = hp.tile([128, 128], bft, tag="s2", name="s2")
                    nc.vector.tensor_copy(s2[:], ps)
                    # transpose S2 tile -> lhsT for dense2
                    trs = trD.tile([128, 128], bft, tag="trs")
                    nc.tensor.matmul(trs[:], lhsT=s2[:],
                                     rhs=ident_b[:], is_transpose=True)
                    s2t = hp.tile([128, 128], bft, tag="s2t")
                    nc.vector.tensor_copy(s2t[:], trs[:])
                    ps2 = psD.tile([128, 384], f32, tag="ps2")
                    nc.tensor.matmul(ps2[:], lhsT=sq_row[0:1, t * 128:(t + 1) * 128],
                                     rhs=brow["b2"][:], start=True, stop=False)
                    nc.tensor.matmul(ps2[:], lhsT=s2t[:],
                                     rhs=W2sb[0][:], start=False, stop=True)
                    h2 = hp.tile([128, 384], bft, tag="h2")
                    act_leaky(h2[:], ps2[:], dinv_pp, t, hp)
                    trp = trD.tile([128, 384], bft, tag="tr")
                    for k in range(3):
                        nc.tensor.matmul(trp[:, k * 128:(k + 1) * 128],
                                         lhsT=h2[:, k * 128:(k + 1) * 128],
                                         rhs=ident_b[:], is_transpose=True)
                    h2t = hp.tile([128, 384], bft, tag="h2t")
                    nc.vector.tensor_copy(h2t[:], trp[:])
                    ps3 = psD.tile([128, 256], f32, tag="ps3")
                    for k in range(3):
                        nc.tensor.matmul(ps3[:], lhsT=h2t[:, k * 128:(k + 1) * 128],
                                         rhs=W3sb[k][:], start=(k == 0), stop=(k == 2))
                    T3t = t3p.tile([128, 256], bft, tag="t3")
                    nc.vector.tensor_scalar(T3t[:], ps3[:], dinv_pp[:, t:t + 1], None,
                                            mybir.AluOpType.mult)
                    nc.sync.dma_start(out=ag_in[2][t * 128:t * 128 + nv(t), :],
                                      in_=T3t[:nv(t), :])
                agg_pass(1, 128, post_d1,
                         after_tile={24: lambda: allgather(2, 0),
                                     TPC - 1: lambda: allgather(2, 1)})

            # ---------- pass E: agg3 -> T4 ----------
            with tc.tile_pool(name="t4p", bufs=4) as t4p:
                def post_e(t, ps):
                    T4t = t4p.tile([128, 256], bft, tag="t4")
                    act_leaky(T4t[:], ps, deginv_pp, t, t4p)
                    nc.sync.dma_start(out=ag_in[3][t * 128:t * 128 + nv(t), :],
                                      in_=T4t[:nv(t), :])
                agg_pass(2, 256, post_e, binit_bias=brow["b3"],
                         after_tile={24: lambda: allgather(3, 0),
                                     TPC - 1: lambda: allgather(3, 1)})

            # ---------- pass F1 (+fused dense4 + dense5) -> out ----------
            with tc.tile_pool(name="hp4", bufs=6) as hp4, \
                 tc.tile_pool(name="op", bufs=4) as op, \
                 tc.tile_pool(name="psF", bufs=1, space="PSUM") as psF, \
                 tc.tile_pool(name="trF", bufs=1, space="PSUM") as trF:
                def post_f1(t, ps):
                    s4 = hp4.tile([128, 256], bft, tag="s4", name="s4")
                    nc.vector.tensor_copy(s4[:], ps)
                    # transpose S4 tile -> 2 lhsT chunks for dense4
                    s4t = hp4.tile([128, 256], bft, tag="s4t")
                    for fk in range(2):
                        trs = trF.tile([128, 128], bft, tag="trs4")
                        nc.tensor.matmul(
                            trs[:],
                            lhsT=s4[:, fk * 128:(fk + 1) * 128],
                            rhs=ident_b[:], is_transpose=True)
                        nc.vector.tensor_copy(s4t[:, fk * 128:(fk + 1) * 128], trs[:])
                    ps4 = psF.tile([128, 384], f32, tag="ps4")
                    nc.tensor.matmul(ps4[:], lhsT=sq_row[0:1, t * 128:(t + 1) * 128],
                                     rhs=brow["b4"][:], start=True, stop=False)
                    for fk in range(2):
                        nc.tensor.matmul(ps4[:],
                                         lhsT=s4t[:, fk * 128:(fk + 1) * 128],
                                         rhs=W4sb[fk][:], start=False, stop=(fk == 1))
                    h4 = hp4.tile([128, 384], bft, tag="h4")
                    act_leaky(h4[:], ps4[:], dinv_pp, t, hp4)
                    trp = trF.tile([128, 384], bft, tag="tr4")
                    for k in range(3):
                        nc.tensor.matmul(trp[:, k * 128:(k + 1) * 128],
                                         lhsT=h4[:, k * 128:(k + 1) * 128],
                                         rhs=ident_b[:], is_transpose=True)
                    h4t = hp4.tile([128, 384], bft, tag="h4t")
                    nc.vector.tensor_copy(h4t[:], trp[:])
                    ps5 = psF.tile([128, 128], f32, tag="ps5")
                    nc.tensor.matmul(ps5[:], lhsT=ones_row[:], rhs=brow["bl"][:],
                                     start=True, stop=False)
                    for k in range(3):
                        nc.tensor.matmul(ps5[:], lhsT=h4t[:, k * 128:(k + 1) * 128],
                                         rhs=Wlsb[k][:], start=False, stop=(k == 2))
                    ot = op.tile([128, 128], f32, tag="o")
                    nc.scalar.activation(ot[:], ps5[:], mybir.ActivationFunctionType.Relu)
                    nc.sync.dma_start(out=out_t[t * 128:t * 128 + nv(t), :],
                                      in_=ot[:nv(t), :])
                agg_pass(3, 256, post_f1)

    nc.compile()
    return nc


def kernel(x, edge_index, W1, b1, W2, b2, W3, b3, W4, b4, Wl, bl,
           trace=False):
    x = np.asarray(x, dtype=np.float32)
    edge_index = np.asarray(edge_index)
    cfg, per_core, wshared = build_host_data(
        x, edge_index, W1, b1, W2, b2, W3, b3, W4, b4, Wl, bl)
    nc = build_program(cfg)
    in_maps = []
    for c in range(NC):
        d = dict(per_core[c])
        m = {"xtt": d["xtt"], "idx": d["idx"], "dl": d["dl"],
             "deg_pp": d["deg_pp"], "deg_row": d["deg_row"]}
        m.update(wshared)
        in_maps.append(m)
    res = run_bass_kernel_spmd(nc, in_maps, core_ids=list(range(NC)),
                               trace=trace)
    out = np.concatenate([res.results[c]["out"] for c in range(NC)], axis=0)
    kernel.last_exec_time_ns = res.exec_time_ns
    kernel.last_results = res
    return out

